# revision 1
# baseline (speedup 1.0000x reference)
"""Trainium2 Bass kernel for CausalTensionGraphLayer.

Math (reference factorization):
  a   = x @ w1[:D] + b1         [T, H]   (H = D/2)
  c   = x @ w1[D:]              [T, H]
  vzb = x @ wv_w + wv_b         [T, D]
  hid_w  = silu(a[t] + c[t-w-1])               (c term is 0 when t-w-1 < 0)
  tau_w  = sigmoid(hid_w @ w2 + b2)
  msg[t] = sum_w tau_w[t] * vzb[t-w-1]         (vzb -> wv_b when t-w-1 < 0)
  y      = x @ merge_w[:D] + msg @ merge_w[D:] + merge_b
  out    = LayerNorm(y) * gamma + beta

Neighbor gathers are row shifts of x, so with zero rows prepended for the
out-of-range halo the same compute path reproduces the reference exactly
(zero x rows give c = 0 and vzb = wv_b).

Sharding: data-parallel over the B*T = 8192 token rows, 1024 own tokens per
core plus a 4-row halo (zeros at batch boundaries, neighbor rows otherwise).
No collectives. Host pre-casts x/weights to bf16 and pre-transposes x so the
device works feature-major (tokens on the free axis -> shifts are free-dim
offsets).

Schedule: phase A (a+c, all token quarters) needs only x/w1 so the PE starts
while wv and the merge weights are still streaming in; phase B (vzb) overlaps
the merge-weight loads; phases C (gating) and D (merge+LN) run per quarter.
Input DMAs are split across the two HWDGE queues (sync, scalar) in the exact
order the PE consumes them.
"""

from contextlib import ExitStack

import numpy as np
import ml_dtypes

import concourse.bass as bass
import concourse.bacc as bacc
import concourse.tile as tile
from concourse import mybir
from concourse.bass_utils import run_bass_kernel_spmd

BF16 = ml_dtypes.bfloat16

B, T, D = 2, 4096, 1024
H = D // 2
W = 4
EPS = 1e-5
NCORES = 8
NTOK = (B * T) // NCORES          # 1024 own tokens per core
HALO = W                          # 4
GRID = NTOK + HALO                # 1028 (halo + own)
NQ = 4                            # token quarters per core
QT = NTOK // NQ                   # 256 own tokens per quarter
QG = QT + HALO                    # 260: shifted-grid cols per quarter
KD = D // 128                     # 8 K-chunks over D
MH = H // 128                     # 4 M-tiles over H
MD = D // 128                     # 8 M-tiles over D
NT = QT // 128                    # 2 token tiles per quarter

FP32 = mybir.dt.float32
I32 = mybir.dt.int32
BF = mybir.dt.bfloat16
AF = mybir.ActivationFunctionType
ALU = mybir.AluOpType
AX = mybir.AxisListType


def build_nc(use_gamma_beta: bool, use_merge_b: bool):
    nc = bacc.Bacc(None, target_bir_lowering=False)

    xT = nc.dram_tensor("xT", [D, GRID], BF, kind="ExternalInput")
    w1a = nc.dram_tensor("w1a", [D, H], BF, kind="ExternalInput")
    w1c = nc.dram_tensor("w1c", [D, H], BF, kind="ExternalInput")
    wv = nc.dram_tensor("wv", [D, D], BF, kind="ExternalInput")
    m1 = nc.dram_tensor("m1", [D, D], BF, kind="ExternalInput")
    m2 = nc.dram_tensor("m2", [D, D], BF, kind="ExternalInput")
    w2rep = nc.dram_tensor("w2rep", [H, 128], BF, kind="ExternalInput")
    b1r = nc.dram_tensor("b1r", [128, MH], FP32, kind="ExternalInput")
    wvbr = nc.dram_tensor("wvbr", [128, MD], FP32, kind="ExternalInput")
    b2r = nc.dram_tensor("b2r", [128, 1], FP32, kind="ExternalInput")
    if use_gamma_beta:
        gam = nc.dram_tensor("gam", [1, D], FP32, kind="ExternalInput")
        bet = nc.dram_tensor("bet", [1, D], FP32, kind="ExternalInput")
    if use_merge_b:
        mbt = nc.dram_tensor("mbt", [1, D], FP32, kind="ExternalInput")
    y = nc.dram_tensor("y", [NTOK, D], FP32, kind="ExternalOutput")

    with tile.TileContext(nc) as tc, ExitStack() as ctx:
        persist = ctx.enter_context(tc.tile_pool(name="persist", bufs=1))
        abpool = ctx.enter_context(tc.tile_pool(name="abpool", bufs=NQ))
        qpool = ctx.enter_context(tc.tile_pool(name="qpool", bufs=2))
        mpool = ctx.enter_context(tc.tile_pool(name="mpool", bufs=4))
        mpool2 = ctx.enter_context(tc.tile_pool(name="mpool2", bufs=2))
        opool = ctx.enter_context(tc.tile_pool(name="opool", bufs=3))
        ps_acc = ctx.enter_context(tc.tile_pool(name="ps_acc", bufs=4, space="PSUM"))
        ps_log = ctx.enter_context(tc.tile_pool(name="ps_log", bufs=1, space="PSUM"))
        ps_y = ctx.enter_context(tc.tile_pool(name="ps_y", bufs=3, space="PSUM"))

        # ---- persistent loads, just-in-time order across both queues ----
        xT_sb = persist.tile([128, KD, GRID], BF, tag="xT")
        w1a_sb = persist.tile([128, KD, H], BF, tag="w1a")
        w1c_sb = persist.tile([128, KD, H], BF, tag="w1c")
        w2rep_sb = persist.tile([128, MH, 128], BF, tag="w2rep")
        wv_sb = persist.tile([128, KD, D], BF, tag="wv")
        m1_sb = persist.tile([128, KD, D], BF, tag="m1")
        m2_sb = persist.tile([128, KD, D], BF, tag="m2")
        b1_sb = persist.tile([128, MH], FP32, tag="b1")
        wvb_sb = persist.tile([128, MD], FP32, tag="wvb")
        b2_sb = persist.tile([128, 1], FP32, tag="b2")
        xT_r = xT.rearrange("(n p) t -> p n t", p=128)
        w1a_r = w1a.rearrange("(n p) m -> p n m", p=128)
        w1c_r = w1c.rearrange("(n p) m -> p n m", p=128)
        w2_r = w2rep.rearrange("(n p) m -> p n m", p=128)
        wv_r = wv.rearrange("(n p) m -> p n m", p=128)
        m1_r = m1.rearrange("(n p) m -> p n m", p=128)
        m2_r = m2.rearrange("(n p) m -> p n m", p=128)
        Q1 = QT + HALO
        # Greedy per-queue byte balancing in PE-consumption order:
        #   sync:   xTq0 | w1c | wv[0:4] | xTq3 | m1
        #   scalar: w1a | xTq1 | wv[4:8] | xTq2 | w2rep+biases | m2
        nc.sync.dma_start(out=xT_sb[:, :, 0:Q1], in_=xT_r[:, :, 0:Q1])
        for mc in range(MH):  # w1a col-chunks so a(q0, m=0) unblocks early
            nc.scalar.dma_start(
                out=w1a_sb[:, :, mc * 128:(mc + 1) * 128],
                in_=w1a_r[:, :, mc * 128:(mc + 1) * 128],
            )
        nc.scalar.dma_start(out=b1_sb, in_=b1r[:, :])
        for mc in range(MH):
            nc.sync.dma_start(
                out=w1c_sb[:, :, mc * 128:(mc + 1) * 128],
                in_=w1c_r[:, :, mc * 128:(mc + 1) * 128],
            )
        nc.scalar.dma_start(
            out=xT_sb[:, :, Q1:Q1 + QT], in_=xT_r[:, :, Q1:Q1 + QT]
        )
        nc.scalar.dma_start(out=wvb_sb, in_=wvbr[:, :])
        for mc in range(MD):
            eng = nc.sync if mc < 4 else nc.scalar
            eng.dma_start(
                out=wv_sb[:, :, mc * 128:(mc + 1) * 128],
                in_=wv_r[:, :, mc * 128:(mc + 1) * 128],
            )
        nc.scalar.dma_start(
            out=xT_sb[:, :, Q1 + QT:Q1 + 2 * QT],
            in_=xT_r[:, :, Q1 + QT:Q1 + 2 * QT],
        )
        nc.sync.dma_start(
            out=xT_sb[:, :, Q1 + 2 * QT:GRID], in_=xT_r[:, :, Q1 + 2 * QT:GRID]
        )
        nc.scalar.dma_start(out=w2rep_sb[:, :, :], in_=w2_r[:, :, :])
        nc.scalar.dma_start(out=b2_sb, in_=b2r[:, :])
        # merge weights last (first needed after phases A+B)
        nc.sync.dma_start(out=m1_sb[:, :, 0:512], in_=m1_r[:, :, 0:512])
        nc.scalar.dma_start(out=m2_sb[:, :, 0:512], in_=m2_r[:, :, 0:512])
        nc.sync.dma_start(out=m1_sb[:, :, 512:D], in_=m1_r[:, :, 512:D])
        nc.scalar.dma_start(out=m2_sb[:, :, 512:D], in_=m2_r[:, :, 512:D])
        magic_sb = persist.tile([128, 1], I32, tag="magic")
        nc.vector.memset(magic_sb, 0x5F3759DF)
        one_i = persist.tile([128, 1], I32, tag="onei")
        nc.vector.memset(one_i, 1)
        if use_gamma_beta:
            gam_sb = persist.tile([128, D], FP32, tag="gam")
            nc.sync.dma_start(out=gam_sb, in_=gam.partition_broadcast(128))
            bet_sb = persist.tile([128, D], FP32, tag="bet")
            nc.sync.dma_start(out=bet_sb, in_=bet.partition_broadcast(128))
        if use_merge_b:
            mb_sb = persist.tile([128, D], FP32, tag="mb")
            nc.sync.dma_start(out=mb_sb, in_=mbt.partition_broadcast(128))

        # ---- phase A: a (own grid) and c (shifted grid), all quarters ----
        aqs, cqs, vzqs = [], [], []
        for q in range(NQ):
            g0 = q * QT
            aq = abpool.tile([128, MH, QT], BF, tag="aq")
            aqs.append(aq)
            cq = abpool.tile([128, MH, QG], BF, tag="cq")
            cqs.append(cq)
            for m in range(MH):
                ps = ps_acc.tile([128, QT], FP32, tag="acc")
                for k in range(KD):
                    nc.tensor.matmul(
                        ps,
                        w1a_sb[:, k, m * 128:(m + 1) * 128],
                        xT_sb[:, k, g0 + HALO:g0 + HALO + QT],
                        start=(k == 0),
                        stop=(k == KD - 1),
                    )
                nc.scalar.activation(
                    out=aq[:, m, :], in_=ps, func=AF.Identity,
                    bias=b1_sb[:, m:m + 1], scale=1.0,
                )
            for m in range(MH):
                ps = ps_acc.tile([128, QG], FP32, tag="acc")
                for k in range(KD):
                    nc.tensor.matmul(
                        ps,
                        w1c_sb[:, k, m * 128:(m + 1) * 128],
                        xT_sb[:, k, g0:g0 + QG],
                        start=(k == 0),
                        stop=(k == KD - 1),
                    )
                nc.scalar.copy(out=cq[:, m, :], in_=ps)
        # ---- phase B: vzb (shifted grid), all quarters -------------------
        for q in range(NQ):
            g0 = q * QT
            vzq = abpool.tile([128, MD, QG], BF, tag="vzq")
            vzqs.append(vzq)
            for m in range(MD):
                ps = ps_acc.tile([128, QG], FP32, tag="acc")
                for k in range(KD):
                    nc.tensor.matmul(
                        ps,
                        wv_sb[:, k, m * 128:(m + 1) * 128],
                        xT_sb[:, k, g0:g0 + QG],
                        start=(k == 0),
                        stop=(k == KD - 1),
                    )
                nc.scalar.activation(
                    out=vzq[:, m, :], in_=ps, func=AF.Identity,
                    bias=wvb_sb[:, m:m + 1], scale=1.0,
                )
        # ---- phase C: gating (hid -> tau -> msg), per quarter ------------
        # silu(z) = z * sigmoid(z) keeps ScalarE in one activation-table set
        # for the whole kernel (silu/sqrt live in different sets; a switch
        # costs ~2.7us). tau comes out of its matmul pre-broadcast across
        # partitions because w2 is replicated over all 128 PE columns.
        msgqs = []
        for q in range(NQ):
            aq, cq, vzq = aqs[q], cqs[q], vzqs[q]
            tauq = qpool.tile([128, W, QT], BF, tag="tauq")
            for p in range(W // 2):
                hs = mpool2.tile([128, MH, 2, QT], BF, tag="hs")
                for wi in range(2):
                    w = 2 * p + wi
                    o = HALO - 1 - w
                    nc.vector.tensor_add(
                        hs[:, :, wi, :], aq, cq[:, :, o:o + QT]
                    )
                sg = mpool2.tile([128, MH, 2, QT], BF, tag="sg")
                nc.scalar.activation(out=sg, in_=hs, func=AF.Sigmoid)
                hss = mpool2.tile([128, MH, 2, QT], BF, tag="hids")
                nc.vector.tensor_mul(hss, hs, sg)
                pl = ps_log.tile([128, 2 * QT], FP32, tag="logit")
                for k in range(MH):
                    nc.tensor.matmul(
                        pl,
                        w2rep_sb[:, k, :],
                        hss[:, k, :, :],
                        start=(k == 0),
                        stop=(k == MH - 1),
                    )
                nc.scalar.activation(
                    out=tauq[:, 2 * p:2 * p + 2, :],
                    in_=pl.rearrange("p (a b) -> p a b", a=2),
                    func=AF.Sigmoid,
                    bias=b2_sb[:, 0:1], scale=1.0,
                )
            # msg = sum_w tau_w * shift(vzb, w+1): fused 3D bf16 ops with tau
            # broadcast over the 8 d-tiles via a step-0 mid dimension.
            msgq = qpool.tile([128, MD, QT], BF, tag="msgq")
            msgqs.append(msgq)

            def tau_b(w, tauq=tauq):
                s = tauq[:, w, :]
                return bass.AP(
                    tensor=s.tensor, offset=s.offset,
                    ap=[s.ap[0], [0, MD], s.ap[1]],
                )

            pw = []
            for w in range(W):
                o = HALO - 1 - w
                pt = mpool.tile([128, MD, QT], BF, tag="pw")
                nc.vector.tensor_mul(pt, tau_b(w), vzq[:, :, o:o + QT])
                pw.append(pt)
                if w == 1:
                    m01 = mpool.tile([128, MD, QT], BF, tag="pw")
                    nc.vector.tensor_add(m01, pw[0], pw[1])
            nc.vector.tensor_add(pw[3], pw[2], pw[3])
            nc.vector.tensor_add(msgq, m01, pw[3])
        # ---- phase D: merge + LayerNorm + store, per quarter -------------
        for q in range(NQ):
            g0 = q * QT
            msgq = msgqs[q]
            srow = mpool.tile([128, NT, 2], FP32, tag="srow")
            sqs = mpool.tile([128, NT, 2], FP32, tag="sqs")
            ysb = []
            for tt in range(NT):
                tok0 = g0 + 128 * tt
                yt = opool.tile([128, D], FP32, tag="ysb")
                ysb.append(yt)
                for half in range(2):
                    n0 = half * 512
                    yps = ps_y.tile([128, 512], FP32, tag="y")
                    for k in range(KD):
                        nc.tensor.matmul(
                            yps,
                            xT_sb[:, k, HALO + tok0:HALO + tok0 + 128],
                            m1_sb[:, k, n0:n0 + 512],
                            start=(k == 0),
                            stop=False,
                        )
                    for k in range(KD):
                        nc.tensor.matmul(
                            yps,
                            msgq[:, k, 128 * tt:128 * tt + 128],
                            m2_sb[:, k, n0:n0 + 512],
                            start=False,
                            stop=(k == KD - 1),
                        )
                    if use_merge_b:
                        nc.vector.tensor_add(yps, yps, mb_sb[:, n0:n0 + 512])
                    # Evict PSUM while collecting LN stats: Copy gives sum(y),
                    # Square gives sum(y^2) — both stay in the sigmoid table
                    # set. 'junk' is a write-only sink for the Square pass.
                    nc.scalar.activation(
                        out=yt[:, n0:n0 + 512], in_=yps, func=AF.Copy,
                        accum_out=srow[:, tt, half:half + 1],
                    )
                    junk = mpool2.tile([128, 512], FP32, tag="junk")
                    nc.scalar.activation(
                        out=junk, in_=yps, func=AF.Square,
                        accum_out=sqs[:, tt, half:half + 1],
                    )
            # LayerNorm finalize for both token tiles at once; rstd via
            # bit-trick seed + 2 Newton steps (keeps sqrt off ScalarE).
            ssum = mpool.tile([128, NT], FP32, tag="ssum")
            nc.vector.reduce_sum(out=ssum, in_=srow, axis=AX.X)
            qsum = mpool.tile([128, NT], FP32, tag="qsum")
            nc.vector.reduce_sum(out=qsum, in_=sqs, axis=AX.X)
            mean = mpool.tile([128, NT], FP32, tag="mean")
            nc.vector.tensor_scalar_mul(mean, ssum, 1.0 / D)
            m2e = mpool.tile([128, NT], FP32, tag="m2e")
            nc.vector.scalar_tensor_tensor(   # mean^2 - eps
                out=m2e, in0=mean, scalar=1.0, in1=mean,
                op0=ALU.mult, op1=ALU.mult,
            )
            nc.vector.tensor_scalar_add(m2e, m2e, -EPS)
            veps = mpool.tile([128, NT], FP32, tag="veps")
            nc.vector.scalar_tensor_tensor(   # q/D - (mean^2 - eps)
                out=veps, in0=qsum, scalar=1.0 / D, in1=m2e,
                op0=ALU.mult, op1=ALU.subtract,
            )
            rbits = mpool.tile([128, NT], I32, tag="rbits")
            nc.vector.tensor_scalar(
                out=rbits, in0=veps.bitcast(I32), scalar1=one_i[:, 0:1],
                scalar2=None, op0=ALU.arith_shift_right,
            )
            nc.vector.tensor_tensor(
                out=rbits, in0=magic_sb.to_broadcast([128, NT]), in1=rbits,
                op=ALU.subtract,
            )
            rstd = rbits.bitcast(FP32)
            for _ in range(2):
                nt1 = mpool.tile([128, NT], FP32, tag="nt1")
                nc.vector.tensor_mul(nt1, rstd, rstd)
                nc.vector.tensor_mul(nt1, nt1, veps)
                nc.vector.tensor_scalar(
                    out=nt1, in0=nt1, scalar1=-0.5, scalar2=1.5,
                    op0=ALU.mult, op1=ALU.add,
                )
                nc.vector.tensor_mul(rstd, rstd, nt1)
            for tt in range(NT):
                tok0 = g0 + 128 * tt
                nc.vector.tensor_scalar(
                    out=ysb[tt], in0=ysb[tt], scalar1=mean[:, tt:tt + 1],
                    scalar2=rstd[:, tt:tt + 1],
                    op0=ALU.subtract, op1=ALU.mult,
                )
                if use_gamma_beta:
                    nc.vector.tensor_mul(ysb[tt], ysb[tt], gam_sb)
                    nc.vector.tensor_add(ysb[tt], ysb[tt], bet_sb)
                nc.sync.dma_start(out=y[tok0:tok0 + 128, :], in_=ysb[tt])
    nc.compile()
    return nc


_CACHE: dict = {}


def _get_nc(use_gamma_beta: bool, use_merge_b: bool):
    key = (use_gamma_beta, use_merge_b)
    if key not in _CACHE:
        _CACHE[key] = build_nc(use_gamma_beta, use_merge_b)
    return _CACHE[key]


def kernel(x, w1, b1, w2, b2, wv_w, wv_b, merge_w, merge_b, gamma, beta):
    x = np.asarray(x, dtype=np.float32)
    w1 = np.asarray(w1, dtype=np.float32)
    b1 = np.asarray(b1, dtype=np.float32)
    w2 = np.asarray(w2, dtype=np.float32)
    b2 = np.asarray(b2, dtype=np.float32)
    wv_w = np.asarray(wv_w, dtype=np.float32)
    wv_b = np.asarray(wv_b, dtype=np.float32)
    merge_w = np.asarray(merge_w, dtype=np.float32)
    merge_b = np.asarray(merge_b, dtype=np.float32)
    gamma = np.asarray(gamma, dtype=np.float32)
    beta = np.asarray(beta, dtype=np.float32)

    use_gamma_beta = not (np.all(gamma == 1.0) and np.all(beta == 0.0))
    use_merge_b = bool(np.any(merge_b != 0.0))
    nc = _get_nc(use_gamma_beta, use_merge_b)

    x2 = x.reshape(B * T, D)
    shared = {
        "w1a": w1[:D].astype(BF16),
        "w1c": w1[D:].astype(BF16),
        "wv": wv_w.astype(BF16),
        "m1": merge_w[:D].astype(BF16),
        "m2": merge_w[D:].astype(BF16),
        "w2rep": np.ascontiguousarray(
            np.broadcast_to(w2.reshape(H, 1), (H, 128))
        ).astype(BF16),
        "b1r": np.ascontiguousarray(b1.reshape(MH, 128).T),
        "wvbr": np.ascontiguousarray(wv_b.reshape(MD, 128).T),
        "b2r": np.full((128, 1), float(b2[0]), np.float32),
    }
    if use_gamma_beta:
        shared["gam"] = gamma.reshape(1, D)
        shared["bet"] = beta.reshape(1, D)
    if use_merge_b:
        shared["mbt"] = merge_b.reshape(1, D)

    in_maps = []
    for c in range(NCORES):
        t0 = c * NTOK
        xs = np.zeros((GRID, D), np.float32)
        xs[HALO:] = x2[t0:t0 + NTOK]
        if t0 % T != 0:  # halo stays inside the same batch element
            xs[:HALO] = x2[t0 - HALO:t0]
        m = dict(shared)
        m["xT"] = np.ascontiguousarray(xs.T).astype(BF16)
        in_maps.append(m)

    res = run_bass_kernel_spmd(nc, in_maps, core_ids=list(range(NCORES)))
    out = np.concatenate([r["y"] for r in res.results], axis=0)
    return out.reshape(B, T, D).astype(np.float32)



# revision 4
# speedup vs baseline: 1.0570x; 1.0570x over previous
"""Trainium2 Bass kernel for CausalTensionGraphLayer.

Math (reference factorization):
  a   = x @ w1[:D] + b1         [T, H]   (H = D/2)
  c   = x @ w1[D:]              [T, H]
  vz  = x @ wv_w + wv_b         [T, D]
  hid_w  = silu(a[t] + c[t-w-1])               (c term is 0 when t-w-1 < 0)
  tau_w  = sigmoid(hid_w @ w2 + b2)
  msg[t] = sum_w tau_w[t] * vz[t-w-1]          (vz -> wv_b when t-w-1 < 0)
  y      = x @ merge_w[:D] + msg @ merge_w[D:] + merge_b
  out    = LayerNorm(y) * gamma + beta

Neighbor gathers are row shifts of x, so with zero rows prepended for the
out-of-range halo the same compute path reproduces the reference exactly.

Sharding: data-parallel over the B*T = 8192 token rows, 1024 own tokens per
core plus a 4-row halo (zeros at batch boundaries, neighbor rows otherwise).
No collectives.

Schedule (phase-major; per-engine queues see work in consumption order):
  A:  a (own tokens) + c (full shifted grid), k-dense chains into 1-bank
      PSUM pairs, paired-m ACT evictions.
  B rounds (r = 0..3): tau(r-1) matmuls | vz slab r | hs(r)=a+shift(c) on
      DVE + hid(r)=Silu(hs) on ACT (one fused call) | msg(r-1) on DVE.
      tau = (1+tanh((z+b2)/2))/2: tanh stored, the +1 is fused into the
      msg multiply (scalar_tensor_tensor), the 1/2 into m2 on the host.
  D:  y = x@m1 + msg@m2h per 128-token tile; LN stats via bn_stats/bn_aggr
      on DVE straight from PSUM while ACT evicts y; rsqrt via bit-trick
      Newton; normalize fused in one tensor_scalar.

DMA: x + w1 on the sync HWDGE queue, remaining weights on the gpsimd SWDGE
queue, both in PE-consumption order; ACT issues no input DMAs so the
activation queue is free for evictions from the first chain on.
"""

from contextlib import ExitStack

import numpy as np
import ml_dtypes

import concourse.bass as bass
import concourse.bacc as bacc
import concourse.tile as tile
from concourse import mybir
from concourse.bass_utils import run_bass_kernel_spmd

BF16 = ml_dtypes.bfloat16

B, T, D = 2, 4096, 1024
H = D // 2
W = 4
EPS = 1e-5
NCORES = 8
NTOK = (B * T) // NCORES          # 1024 own tokens per core
HALO = W                          # 4
GRID = NTOK + HALO                # 1028 (halo + own)
NQ = 4                            # token quarters per core
QT = NTOK // NQ                   # 256 own tokens per quarter
KD = D // 128                     # 8 K-chunks over D
MH = H // 128                     # 4 M-tiles over H
MD = D // 128                     # 8 M-tiles over D
NT = QT // 128                    # 2 token tiles per quarter

FP32 = mybir.dt.float32
I32 = mybir.dt.int32
BF = mybir.dt.bfloat16
AF = mybir.ActivationFunctionType
ALU = mybir.AluOpType
AX = mybir.AxisListType


def build_nc(flags):
    use_gamma_beta, use_merge_b, use_b1, use_wvb, b2_half = flags
    nc = bacc.Bacc(None, target_bir_lowering=False)

    xT = nc.dram_tensor("xT", [D, GRID], BF, kind="ExternalInput")
    w1a = nc.dram_tensor("w1a", [D, H], BF, kind="ExternalInput")
    w1c = nc.dram_tensor("w1c", [D, H], BF, kind="ExternalInput")
    wv = nc.dram_tensor("wv", [D, D], BF, kind="ExternalInput")
    m1 = nc.dram_tensor("m1", [D, D], BF, kind="ExternalInput")
    m2 = nc.dram_tensor("m2", [D, D], BF, kind="ExternalInput")
    w2rep = nc.dram_tensor("w2rep", [H, 128], BF, kind="ExternalInput")
    if use_b1:
        b1r = nc.dram_tensor("b1r", [128, MH], FP32, kind="ExternalInput")
    if use_wvb:
        wvbr = nc.dram_tensor("wvbr", [128, MD], FP32, kind="ExternalInput")
    if use_gamma_beta:
        gam = nc.dram_tensor("gam", [1, D], FP32, kind="ExternalInput")
        bet = nc.dram_tensor("bet", [1, D], FP32, kind="ExternalInput")
    if use_merge_b:
        mbt = nc.dram_tensor("mbt", [1, D], FP32, kind="ExternalInput")
    y = nc.dram_tensor("y", [NTOK, D], FP32, kind="ExternalOutput")

    with tile.TileContext(nc) as tc, ExitStack() as ctx:
        persist = ctx.enter_context(tc.tile_pool(name="persist", bufs=1))
        hspool = ctx.enter_context(tc.tile_pool(name="hspool", bufs=2))
        hsspool = ctx.enter_context(tc.tile_pool(name="hsspool", bufs=5))
        pwpool = ctx.enter_context(tc.tile_pool(name="pwpool", bufs=4))
        opool = ctx.enter_context(tc.tile_pool(name="opool", bufs=3))
        lnpool = ctx.enter_context(tc.tile_pool(name="lnpool", bufs=2))
        ps_gate = ctx.enter_context(tc.tile_pool(name="ps_gate", bufs=3, space="PSUM"))
        ps_log = ctx.enter_context(tc.tile_pool(name="ps_log", bufs=1, space="PSUM"))
        ps_y = ctx.enter_context(tc.tile_pool(name="ps_y", bufs=2, space="PSUM"))

        # ---- persistent inputs; sync queue: x + w1 (gate path, needed
        # first), gpsimd SWDGE queue: everything else, in consumption order.
        xT_sb = persist.tile([128, KD, GRID], BF, tag="xT")
        w1a_sb = persist.tile([128, KD, H], BF, tag="w1a")
        w1c_sb = persist.tile([128, KD, H], BF, tag="w1c")
        w2rep_sb = persist.tile([128, MH, 128], BF, tag="w2rep")
        wv_sb = persist.tile([128, KD, D], BF, tag="wv")
        m1_sb = persist.tile([128, KD, D], BF, tag="m1")
        m2_sb = persist.tile([128, KD, D], BF, tag="m2")
        xT_r = xT.rearrange("(n p) t -> p n t", p=128)
        w1a_r = w1a.rearrange("(n p) m -> p n m", p=128)
        w1c_r = w1c.rearrange("(n p) m -> p n m", p=128)
        w2_r = w2rep.rearrange("(n p) m -> p n m", p=128)
        wv_r = wv.rearrange("(n p) m -> p n m", p=128)
        m1_r = m1.rearrange("(n p) m -> p n m", p=128)
        m2_r = m2.rearrange("(n p) m -> p n m", p=128)

        nc.sync.dma_start(out=xT_sb[:, :, 0:260], in_=xT_r[:, :, 0:260])
        for mc in range(MH):  # w1a in m-chunks so slab 0 unblocks early
            nc.sync.dma_start(
                out=w1a_sb[:, :, mc * 128:(mc + 1) * 128],
                in_=w1a_r[:, :, mc * 128:(mc + 1) * 128],
            )
        for mc in range(MH):
            nc.sync.dma_start(
                out=w1c_sb[:, :, mc * 128:(mc + 1) * 128],
                in_=w1c_r[:, :, mc * 128:(mc + 1) * 128],
            )
        nc.sync.dma_start(out=xT_sb[:, :, 260:516], in_=xT_r[:, :, 260:516])
        nc.sync.dma_start(out=xT_sb[:, :, 516:772], in_=xT_r[:, :, 516:772])
        nc.sync.dma_start(out=xT_sb[:, :, 772:GRID], in_=xT_r[:, :, 772:GRID])

        for mc in range(MD):
            nc.gpsimd.dma_start(
                out=wv_sb[:, :, mc * 128:(mc + 1) * 128],
                in_=wv_r[:, :, mc * 128:(mc + 1) * 128],
            )
        nc.gpsimd.dma_start(out=w2rep_sb[:, :, :], in_=w2_r[:, :, :])
        nc.gpsimd.dma_start(out=m1_sb[:, :, 0:512], in_=m1_r[:, :, 0:512])
        nc.gpsimd.dma_start(out=m1_sb[:, :, 512:D], in_=m1_r[:, :, 512:D])
        nc.gpsimd.dma_start(out=m2_sb[:, :, 0:512], in_=m2_r[:, :, 0:512])
        nc.gpsimd.dma_start(out=m2_sb[:, :, 512:D], in_=m2_r[:, :, 512:D])

        if use_b1:
            b1_sb = persist.tile([128, MH], FP32, tag="b1")
            nc.gpsimd.dma_start(out=b1_sb, in_=b1r[:, :])
        if use_wvb:
            wvb_sb = persist.tile([128, MD], FP32, tag="wvb")
            nc.gpsimd.dma_start(out=wvb_sb, in_=wvbr[:, :])
        if use_gamma_beta:
            gam_sb = persist.tile([128, D], FP32, tag="gam")
            nc.gpsimd.dma_start(out=gam_sb, in_=gam.partition_broadcast(128))
            bet_sb = persist.tile([128, D], FP32, tag="bet")
            nc.gpsimd.dma_start(out=bet_sb, in_=bet.partition_broadcast(128))
        if use_merge_b:
            mb_sb = persist.tile([128, D], FP32, tag="mb")
            nc.gpsimd.dma_start(out=mb_sb, in_=mbt.partition_broadcast(128))

        magic_sb = persist.tile([128, 1], I32, tag="magic")
        nc.vector.memset(magic_sb, 0x5F3759DF)
        one_i = persist.tile([128, 1], I32, tag="onei")
        nc.vector.memset(one_i, 1)

        # Full-grid gate tensors (bf16, feature-major: tokens on free axis).
        a_g = persist.tile([128, MH, NTOK], BF, tag="a_g")
        c_g = persist.tile([128, MH, GRID], BF, tag="c_g")
        vz_g = persist.tile([128, MD, GRID], BF, tag="vz_g")
        tauqs = [
            persist.tile([128, W, QT], BF, tag=f"tau{q}", name=f"tau{q}")
            for q in range(NQ)
        ]
        msgqs = [
            persist.tile([128, MD, QT], BF, tag=f"msg{q}", name=f"msg{q}")
            for q in range(NQ)
        ]

        def gate_chain(wsb, dst, src0, dst0, n, mp, bias_sb):
            """k-dense chain for 2 m-tiles of x@wsb over grid cols
            [src0, src0+n), evicted into dst[:, 2mp:2mp+2, dst0:dst0+n]."""
            ps = ps_gate.tile([128, 2, QT], FP32, tag="g")
            for j in range(2):
                m = 2 * mp + j
                for k in range(KD):
                    nc.tensor.matmul(
                        ps[:, j, 0:n],
                        wsb[:, k, m * 128:(m + 1) * 128],
                        xT_sb[:, k, src0:src0 + n],
                        start=(k == 0),
                        stop=(k == KD - 1),
                    )
            if bias_sb is None:
                nc.scalar.activation(
                    out=dst[:, 2 * mp:2 * mp + 2, dst0:dst0 + n],
                    in_=ps[:, :, 0:n], func=AF.Copy,
                )
            else:
                for j in range(2):
                    m = 2 * mp + j
                    nc.scalar.activation(
                        out=dst[:, m, dst0:dst0 + n],
                        in_=ps[:, j, 0:n], func=AF.Identity,
                        bias=bias_sb[:, m:m + 1], scale=1.0,
                    )

        # ---- phase A: a (own tokens) and c (shifted grid) ----------------
        b1s = b1_sb if use_b1 else None
        for s in range(NQ):
            for mp in range(MH // 2):
                gate_chain(w1a_sb, a_g, HALO + QT * s, QT * s, QT, mp, b1s)
            for mp in range(MH // 2):
                gate_chain(w1c_sb, c_g, QT * s, QT * s, QT, mp, None)
        for mp in range(MH // 2):  # c tail cols [1024, 1028)
            gate_chain(w1c_sb, c_g, NQ * QT, NQ * QT, HALO, mp, None)

        # hs_w[t] = a[t] + c[t-w-1]: token t-w-1 lives at grid col t+o with
        # o = HALO-1-w; a_g is own-token-indexed, c_g/vz_g grid-indexed.
        hsss = [[None, None] for _ in range(NQ)]

        def c1_block(q):
            g0 = q * QT
            for p in range(W // 2):
                hs = hspool.tile([128, MH, 2, QT], BF, tag="hs")
                for wi in range(2):
                    w = 2 * p + wi
                    o = HALO - 1 - w
                    nc.vector.tensor_add(
                        hs[:, :, wi, :],
                        a_g[:, :, g0:g0 + QT],
                        c_g[:, :, g0 + o:g0 + o + QT],
                    )
                hss = hsspool.tile([128, MH, 2, QT], BF, tag="hss")
                nc.scalar.activation(out=hss, in_=hs, func=AF.Silu)
                hsss[q][p] = hss

        def tau_block(q):
            tq = tauqs[q]
            for p in range(W // 2):
                pl = ps_log.tile([128, 2 * QT], FP32, tag="logit")
                for k in range(MH):
                    nc.tensor.matmul(
                        pl,
                        w2rep_sb[:, k, :],
                        hsss[q][p][:, k, :, :],
                        start=(k == 0),
                        stop=(k == MH - 1),
                    )
                nc.scalar.activation(
                    out=tq[:, 2 * p:2 * p + 2, :],
                    in_=pl.rearrange("p (a b) -> p a b", a=2),
                    func=AF.Tanh, scale=0.5, bias=float(b2_half),
                )

        def c3_block(q):
            g0 = q * QT
            tq = tauqs[q]

            def tau_b(w):
                s = tq[:, w, :]
                return bass.AP(
                    tensor=s.tensor, offset=s.offset,
                    ap=[s.ap[0], [0, MD], s.ap[1]],
                )

            pw = []
            for w in range(W):
                o = HALO - 1 - w
                pt = pwpool.tile([128, MD, QT], BF, tag="pw")
                nc.vector.scalar_tensor_tensor(
                    out=pt, in0=tau_b(w), scalar=1.0,
                    in1=vz_g[:, :, g0 + o:g0 + o + QT],
                    op0=ALU.add, op1=ALU.mult,
                )
                pw.append(pt)
                if w == 1:
                    nc.vector.tensor_add(pw[1], pw[0], pw[1])
            nc.vector.tensor_add(pw[3], pw[2], pw[3])
            nc.vector.tensor_add(msgqs[q], pw[1], pw[3])

        # ---- phase B rounds: tau(r-1) | vz slab r | C1(r) | msg(r-1) -----
        wvbs = wvb_sb if use_wvb else None
        for r in range(NQ):
            if r > 0:
                tau_block(r - 1)
            for mp in range(MD // 2):
                gate_chain(wv_sb, vz_g, QT * r, QT * r, QT, mp, wvbs)
            c1_block(r)
            if r > 0:
                c3_block(r - 1)
        for mp in range(MD // 2):  # vz tail cols [1024, 1028)
            gate_chain(wv_sb, vz_g, NQ * QT, NQ * QT, HALO, mp, wvbs)
        tau_block(NQ - 1)
        c3_block(NQ - 1)

        # ---- phase D: merge + LayerNorm + store, per quarter -------------
        for q in range(NQ):
            g0 = q * QT
            msgq = msgqs[q]
            mvq = lnpool.tile([128, NT, 2], FP32, tag="mvq")
            ysb = []
            for tt in range(NT):
                tok0 = g0 + 128 * tt
                yps = ps_y.tile([128, 1024], FP32, tag="y")
                for half in range(2):
                    n0 = half * 512
                    for k in range(KD):
                        nc.tensor.matmul(
                            yps[:, n0:n0 + 512],
                            xT_sb[:, k, HALO + tok0:HALO + tok0 + 128],
                            m1_sb[:, k, n0:n0 + 512],
                            start=(k == 0),
                            stop=False,
                        )
                    for k in range(KD):
                        nc.tensor.matmul(
                            yps[:, n0:n0 + 512],
                            msgq[:, k, 128 * tt:128 * tt + 128],
                            m2_sb[:, k, n0:n0 + 512],
                            start=False,
                            stop=(k == KD - 1),
                        )
                if use_merge_b:
                    nc.vector.tensor_add(yps, yps, mb_sb)
                # LN stats on DVE straight from PSUM; y evict on ACT.
                bst = lnpool.tile([128, 2, 6], FP32, tag="bst")
                ypsv = yps.rearrange("p (a b) -> p a b", a=2)
                for sub in range(2):
                    nc.vector.bn_stats(out=bst[:, sub, :], in_=ypsv[:, sub, :])
                nc.vector.bn_aggr(out=mvq[:, tt, :], in_=bst)
                yt = opool.tile([128, D], FP32, tag="ysb")
                ysb.append(yt)
                nc.scalar.activation(out=yt, in_=yps, func=AF.Copy)
            # rstd = rsqrt(var + eps) via bit-trick seed + 2 Newton steps.
            veps = lnpool.tile([128, NT], FP32, tag="veps")
            var_ap = bass.AP(
                tensor=mvq.tensor, offset=mvq.offset + 1,
                ap=[mvq.ap[0], [2, NT]],
            )
            nc.vector.tensor_scalar_add(veps, var_ap, EPS)
            rbits = lnpool.tile([128, NT], I32, tag="rbits")
            nc.vector.tensor_scalar(
                out=rbits, in0=veps.bitcast(I32), scalar1=one_i[:, 0:1],
                scalar2=None, op0=ALU.arith_shift_right,
            )
            nc.vector.tensor_tensor(
                out=rbits, in0=magic_sb.to_broadcast([128, NT]), in1=rbits,
                op=ALU.subtract,
            )
            rstd = rbits.bitcast(FP32)
            for _ in range(2):
                nt1 = lnpool.tile([128, NT], FP32, tag="nt1")
                nc.vector.tensor_mul(nt1, rstd, rstd)
                nc.vector.tensor_mul(nt1, nt1, veps)
                nc.vector.tensor_scalar(
                    out=nt1, in0=nt1, scalar1=-0.5, scalar2=1.5,
                    op0=ALU.mult, op1=ALU.add,
                )
                nc.vector.tensor_mul(rstd, rstd, nt1)
            for tt in range(NT):
                tok0 = g0 + 128 * tt
                nc.vector.tensor_scalar(
                    out=ysb[tt], in0=ysb[tt], scalar1=mvq[:, tt, 0:1],
                    scalar2=rstd[:, tt:tt + 1],
                    op0=ALU.subtract, op1=ALU.mult,
                )
                if use_gamma_beta:
                    nc.vector.tensor_mul(ysb[tt], ysb[tt], gam_sb)
                    nc.vector.tensor_add(ysb[tt], ysb[tt], bet_sb)
                nc.sync.dma_start(out=y[tok0:tok0 + 128, :], in_=ysb[tt])
    nc.compile()
    return nc


_CACHE: dict = {}


def _get_nc(flags):
    if flags not in _CACHE:
        _CACHE[flags] = build_nc(flags)
    return _CACHE[flags]


def kernel(x, w1, b1, w2, b2, wv_w, wv_b, merge_w, merge_b, gamma, beta):
    x = np.asarray(x, dtype=np.float32)
    w1 = np.asarray(w1, dtype=np.float32)
    b1 = np.asarray(b1, dtype=np.float32)
    w2 = np.asarray(w2, dtype=np.float32)
    b2 = np.asarray(b2, dtype=np.float32)
    wv_w = np.asarray(wv_w, dtype=np.float32)
    wv_b = np.asarray(wv_b, dtype=np.float32)
    merge_w = np.asarray(merge_w, dtype=np.float32)
    merge_b = np.asarray(merge_b, dtype=np.float32)
    gamma = np.asarray(gamma, dtype=np.float32)
    beta = np.asarray(beta, dtype=np.float32)

    use_gamma_beta = not (np.all(gamma == 1.0) and np.all(beta == 0.0))
    use_merge_b = bool(np.any(merge_b != 0.0))
    use_b1 = bool(np.any(b1 != 0.0))
    use_wvb = bool(np.any(wv_b != 0.0))
    b2_half = 0.5 * float(b2[0])
    flags = (use_gamma_beta, use_merge_b, use_b1, use_wvb, b2_half)
    nc = _get_nc(flags)

    x2 = x.reshape(B * T, D)
    shared = {
        "w1a": w1[:D].astype(BF16),
        "w1c": w1[D:].astype(BF16),
        "wv": wv_w.astype(BF16),
        "m1": merge_w[:D].astype(BF16),
        "m2": (0.5 * merge_w[D:]).astype(BF16),
        "w2rep": np.ascontiguousarray(
            np.broadcast_to(w2.reshape(H, 1), (H, 128))
        ).astype(BF16),
    }
    if use_b1:
        shared["b1r"] = np.ascontiguousarray(b1.reshape(MH, 128).T)
    if use_wvb:
        shared["wvbr"] = np.ascontiguousarray(wv_b.reshape(MD, 128).T)
    if use_gamma_beta:
        shared["gam"] = gamma.reshape(1, D)
        shared["bet"] = beta.reshape(1, D)
    if use_merge_b:
        shared["mbt"] = merge_b.reshape(1, D)

    in_maps = []
    for c in range(NCORES):
        t0 = c * NTOK
        xs = np.zeros((GRID, D), np.float32)
        xs[HALO:] = x2[t0:t0 + NTOK]
        if t0 % T != 0:  # halo stays inside the same batch element
            xs[:HALO] = x2[t0 - HALO:t0]
        m = dict(shared)
        m["xT"] = np.ascontiguousarray(xs.T).astype(BF16)
        in_maps.append(m)

    res = run_bass_kernel_spmd(nc, in_maps, core_ids=list(range(NCORES)))
    out = np.concatenate([r["y"] for r in res.results], axis=0)
    return out.reshape(B, T, D).astype(np.float32)


# revision 19
# speedup vs baseline: 1.2423x; 1.1753x over previous
"""Trainium2 Bass kernel for CausalTensionGraphLayer.

Math (reference factorization, with the value path folded through merge):
  a   = x @ w1[:D] + b1                [T, H]   (H = D/2)
  c   = x @ w1[D:]                     [T, H]
  u   = x @ wvm + k2,  wvm = wv_w @ (m2/2),  k2 = wv_b @ (m2/2)
  hid_w  = silu(a[t] + c[t-w-1])                (c, u are 0/k2 for t-w-1 < 0)
  tau2_w = 1 + tanh((hid_w @ w2 + b2)/2)        (= 2*sigmoid(logit))
  y      = x @ m1 + sum_w tau2_w[t] * u[t-w-1]  (+ merge_b)
  out    = LayerNorm(y) * gamma + beta

Folding m2 into the value projection on the host removes the entire
msg @ m2 matmul block (1M MACs/token) and the m2 weight load; the whole
kernel then runs feature-major (tokens on the free axis), msg is added
into the y PSUM with a 1-step identity matmul, and LayerNorm statistics
are computed with ones-matmul partition reductions on the PE.

Sharding: data-parallel over the B*T = 8192 token rows, 1024 own tokens
per core plus a 4-row halo (zeros at batch boundaries).  No collectives.
The output leaves the device feature-major [D, NTOK] (bf16 values cast
to fp32 by the SWDGE store); the host transposes.

Schedule: A (a, then c) -> B rounds r: tau(r-1) | u slab r | hs/silu(r) |
msg(r-1) -> D per quarter (y chains + stats + LN + store).  Input DMA:
x + w1 on sync (HWDGE) at full bandwidth; remaining weights on the
gpsimd SWDGE queue, gated behind the first a-eviction.
"""

from contextlib import ExitStack

import numpy as np
import ml_dtypes

import concourse.bass as bass
import concourse.bacc as bacc
import concourse.tile as tile
from concourse import mybir
from concourse.bass_utils import run_bass_kernel_spmd

BF16 = ml_dtypes.bfloat16

B, T, D = 2, 4096, 1024
H = D // 2
W = 4
EPS = 1e-5
NCORES = 8
NTOK = (B * T) // NCORES          # 1024 own tokens per core
HALO = W                          # 4
GRID = NTOK + HALO                # 1028 (halo + own)
NQ = 4                            # token quarters per core
QT = NTOK // NQ                   # 256 own tokens per quarter
KD = D // 128                     # 8 K-chunks over D
MH = H // 128                     # 4 M-tiles over H
MD = D // 128                     # 8 M-tiles over D

FP32 = mybir.dt.float32
I32 = mybir.dt.int32
BF = mybir.dt.bfloat16
AF = mybir.ActivationFunctionType
ALU = mybir.AluOpType


def build_nc(flags):
    use_gamma_beta, use_merge_b, use_b1, use_k2, b2_half = flags
    nc = bacc.Bacc(None, target_bir_lowering=False)

    xT = nc.dram_tensor("xT", [D, GRID], BF, kind="ExternalInput")
    w1a = nc.dram_tensor("w1a", [D, H], BF, kind="ExternalInput")
    w1c = nc.dram_tensor("w1c", [D, H], BF, kind="ExternalInput")
    wvm = nc.dram_tensor("wvm", [D, D], BF, kind="ExternalInput")
    m1 = nc.dram_tensor("m1", [D, D], BF, kind="ExternalInput")
    w2rep = nc.dram_tensor("w2rep", [H, 128], BF, kind="ExternalInput")
    ident = nc.dram_tensor("ident", [128, 128], BF, kind="ExternalInput")
    if use_b1:
        b1r = nc.dram_tensor("b1r", [128, MH], FP32, kind="ExternalInput")
    if use_k2:
        k2r = nc.dram_tensor("k2r", [128, MD], FP32, kind="ExternalInput")
    if use_gamma_beta:
        gamr = nc.dram_tensor("gamr", [128, MD], FP32, kind="ExternalInput")
        betr = nc.dram_tensor("betr", [128, MD], FP32, kind="ExternalInput")
    if use_merge_b:
        mbr = nc.dram_tensor("mbr", [128, MD], FP32, kind="ExternalInput")
    y = nc.dram_tensor("y", [D, NTOK], FP32, kind="ExternalOutput")

    with tile.TileContext(nc) as tc, ExitStack() as ctx:
        persist = ctx.enter_context(tc.tile_pool(name="persist", bufs=1))
        hspool = ctx.enter_context(tc.tile_pool(name="hspool", bufs=2))
        hsspool = ctx.enter_context(tc.tile_pool(name="hsspool", bufs=4))
        pwpool = ctx.enter_context(tc.tile_pool(name="pwpool", bufs=4))
        ypool = ctx.enter_context(tc.tile_pool(name="ypool", bufs=3))
        lnpool = ctx.enter_context(tc.tile_pool(name="lnpool", bufs=1))
        ps_gate = ctx.enter_context(tc.tile_pool(name="ps_gate", bufs=2, space="PSUM"))
        ps_log = ctx.enter_context(tc.tile_pool(name="ps_log", bufs=1, space="PSUM"))
        ps_y = ctx.enter_context(tc.tile_pool(name="ps_y", bufs=3, space="PSUM"))

        xT_sb = persist.tile([128, KD, GRID], BF, tag="xT")
        w1a_sb = persist.tile([128, KD, H], BF, tag="w1a")
        w1c_sb = persist.tile([128, KD, H], BF, tag="w1c")
        w2rep_sb = persist.tile([128, MH, 128], BF, tag="w2rep")
        wvm_sb = persist.tile([128, KD, D], BF, tag="wvm")
        m1_sb = persist.tile([128, KD, D], BF, tag="m1")
        ident_sb = persist.tile([128, 128], BF, tag="ident")
        ones_sb = persist.tile([128, 128], BF, tag="ones")
        nc.vector.memset(ones_sb, 1.0)
        xT_r = xT.rearrange("(n p) t -> p n t", p=128)
        w1a_r = w1a.rearrange("(n p) m -> p n m", p=128)
        w1c_r = w1c.rearrange("(n p) m -> p n m", p=128)
        w2_r = w2rep.rearrange("(n p) m -> p n m", p=128)
        wvm_r = wvm.rearrange("(n p) m -> p n m", p=128)
        m1_r = m1.rearrange("(n p) m -> p n m", p=128)
        y_r = y.rearrange("(n p) t -> p n t", p=128)

        # Sync queue: x + w1 in consumption order, full HBM bandwidth
        # (the SWDGE stream below is gated behind the first a-eviction).
        nc.sync.dma_start(out=xT_sb[:, :, 0:260], in_=xT_r[:, :, 0:260])
        nc.sync.dma_start(out=w1a_sb[:, :, 0:256], in_=w1a_r[:, :, 0:256])
        nc.sync.dma_start(out=w1a_sb[:, :, 256:H], in_=w1a_r[:, :, 256:H])
        nc.sync.dma_start(out=xT_sb[:, :, 260:516], in_=xT_r[:, :, 260:516])
        nc.sync.dma_start(out=xT_sb[:, :, 516:772], in_=xT_r[:, :, 516:772])
        nc.sync.dma_start(out=xT_sb[:, :, 772:GRID], in_=xT_r[:, :, 772:GRID])
        nc.sync.dma_start(out=w1c_sb[:, :, 0:256], in_=w1c_r[:, :, 0:256])
        nc.sync.dma_start(out=w1c_sb[:, :, 256:H], in_=w1c_r[:, :, 256:H])
        if use_b1:
            b1_sb = persist.tile([128, MH], FP32, tag="b1")
            nc.sync.dma_start(out=b1_sb, in_=b1r[:, :])
        if use_k2:
            k2_sb = persist.tile([128, MD], FP32, tag="k2")
            nc.sync.dma_start(out=k2_sb, in_=k2r[:, :])
        if use_gamma_beta:
            gam_sb = persist.tile([128, MD], FP32, tag="gam")
            nc.sync.dma_start(out=gam_sb, in_=gamr[:, :])
            bet_sb = persist.tile([128, MD], FP32, tag="bet")
            nc.sync.dma_start(out=bet_sb, in_=betr[:, :])
        if use_merge_b:
            mb_sb = persist.tile([128, MD], FP32, tag="mb")
            nc.sync.dma_start(out=mb_sb, in_=mbr[:, :])

        magic_sb = persist.tile([128, 1], I32, tag="magic")
        nc.vector.memset(magic_sb, 0x5F3759DF)
        one_i = persist.tile([128, 1], I32, tag="onei")
        nc.vector.memset(one_i, 1)

        # Full-grid gate tensors (bf16, feature-major: tokens on free axis).
        a_g = persist.tile([128, MH, NTOK], BF, tag="a_g")
        c_g = persist.tile([128, MH, GRID], BF, tag="c_g")
        u_g = persist.tile([128, MD, GRID], BF, tag="u_g")
        tauqs = [
            persist.tile([128, W, QT], BF, tag=f"tau{q}", name=f"tau{q}")
            for q in range(NQ)
        ]
        msgqs = [
            persist.tile([128, MD, QT], BF, tag=f"msg{q}", name=f"msg{q}")
            for q in range(NQ)
        ]

        def gate_chain(wsb, dst, src0, dst0, n, mp, bias_sb):
            """k-dense chain for 2 m-tiles of x@wsb over grid cols
            [src0, src0+n), evicted into dst[:, 2mp:2mp+2, dst0:dst0+n].
            The 512-wide psum rows keep each j-chain inside one PSUM bank
            (a matmul output cannot cross the 2KB bank boundary)."""
            ps = ps_gate.tile([128, 2, 512], FP32, tag="g")
            for j in range(2):
                m = 2 * mp + j
                for k in range(KD):
                    nc.tensor.matmul(
                        ps[:, j, 0:n],
                        wsb[:, k, m * 128:(m + 1) * 128],
                        xT_sb[:, k, src0:src0 + n],
                        start=(k == 0),
                        stop=(k == KD - 1),
                    )
            if bias_sb is None:
                nc.scalar.activation(
                    out=dst[:, 2 * mp:2 * mp + 2, dst0:dst0 + n],
                    in_=ps[:, :, 0:n], func=AF.Copy,
                )
            else:
                for j in range(2):
                    m = 2 * mp + j
                    nc.scalar.activation(
                        out=dst[:, m, dst0:dst0 + n],
                        in_=ps[:, j, 0:n], func=AF.Identity,
                        bias=bias_sb[:, m:m + 1], scale=1.0,
                    )

        # ---- phase A: a (own tokens) then c (shifted grid) ---------------
        b1s = b1_sb if use_b1 else None
        gpsimd_gate = None
        for s in range(NQ):
            for mp in range(MH // 2):
                gate_chain(w1a_sb, a_g, HALO + QT * s, QT * s, QT, mp, b1s)
            if s == 0 and gpsimd_gate is None:
                # Gate the SWDGE weight stream behind the first a-eviction.
                gpsimd_gate = persist.tile([128, 8], BF, tag="gate")
                nc.gpsimd.tensor_copy(gpsimd_gate, a_g[:, 0, 0:8])
                for mc in range(2):
                    nc.gpsimd.dma_start(
                        out=wvm_sb[:, :, mc * 512:(mc + 1) * 512],
                        in_=wvm_r[:, :, mc * 512:(mc + 1) * 512],
                    )
                nc.gpsimd.dma_start(out=w2rep_sb[:, :, :], in_=w2_r[:, :, :])
                nc.gpsimd.dma_start(out=m1_sb[:, :, 0:512], in_=m1_r[:, :, 0:512])
                nc.gpsimd.dma_start(out=m1_sb[:, :, 512:D], in_=m1_r[:, :, 512:D])
                nc.gpsimd.dma_start(out=ident_sb, in_=ident[:, :])
        for s in range(NQ):
            for mp in range(MH // 2):
                gate_chain(w1c_sb, c_g, QT * s, QT * s, 260, mp, None)

        # hs_w[t] = a[t] + c[t-w-1]: token t-w-1 lives at grid col t+o with
        # o = HALO-1-w; a_g is own-token-indexed, c_g/u_g grid-indexed.
        hsss = [[None, None] for _ in range(NQ)]

        def c1_block(q):
            g0 = q * QT
            for p in range(W // 2):
                hs = hspool.tile([128, MH, 2, QT], BF, tag="hs")
                for wi in range(2):
                    w = 2 * p + wi
                    o = HALO - 1 - w
                    nc.vector.tensor_add(
                        hs[:, :, wi, :],
                        a_g[:, :, g0:g0 + QT],
                        c_g[:, :, g0 + o:g0 + o + QT],
                    )
                hss = hsspool.tile([128, MH, 2, QT], BF, tag="hss")
                nc.scalar.activation(out=hss, in_=hs, func=AF.Silu)
                hsss[q][p] = hss

        def tau_block(q):
            tq = tauqs[q]
            for p in range(W // 2):
                pl = ps_log.tile([128, 512], FP32, tag="logit")
                for k in range(MH):
                    nc.tensor.matmul(
                        pl,
                        w2rep_sb[:, k, :],
                        hsss[q][p][:, k, :, :],
                        start=(k == 0),
                        stop=(k == MH - 1),
                    )
                nc.scalar.activation(
                    out=tq[:, 2 * p:2 * p + 2, :],
                    in_=pl.rearrange("p (a b) -> p a b", a=2),
                    func=AF.Tanh, scale=0.5, bias=float(b2_half),
                )
            # tq <- tanh+1 = 2*tau (the 1/2 lives in wvm).
            nc.vector.tensor_scalar_add(tq, tq, 1.0)

        def bcast(ap2d, nmid):
            """[128, N] AP -> [128, (0-step nmid), N] broadcast AP."""
            return bass.AP(
                tensor=ap2d.tensor, offset=ap2d.offset,
                ap=[ap2d.ap[0], [0, nmid], ap2d.ap[1]],
            )

        def c3_block(q):
            g0 = q * QT
            tq = tauqs[q]
            pw = []
            for w in range(W):
                o = HALO - 1 - w
                pt = pwpool.tile([128, MD, QT], BF, tag="pw")
                nc.vector.tensor_mul(
                    pt, bcast(tq[:, w, :], MD), u_g[:, :, g0 + o:g0 + o + QT]
                )
                pw.append(pt)
                if w == 1:
                    nc.vector.tensor_add(pw[1], pw[0], pw[1])
            nc.vector.tensor_add(pw[3], pw[2], pw[3])
            nc.vector.tensor_add(msgqs[q], pw[1], pw[3])

        # ---- phase B rounds: tau(r-1) | u slab r | C1(r) | msg(r-1) ------
        k2s = k2_sb if use_k2 else None
        for r in range(NQ):
            if r > 0:
                tau_block(r - 1)
            for mp in range(MD // 2):
                gate_chain(wvm_sb, u_g, QT * r, QT * r, 260, mp, k2s)
            c1_block(r)
            if r > 0:
                c3_block(r - 1)
        tau_block(NQ - 1)
        c3_block(NQ - 1)

        # ---- phase D: y = m1^T x + msg (feature-major), LN, store --------
        # Pipelined per quarter: stats/LN/store of quarter q are emitted
        # after the y-chains of quarter q+1 so the PE never waits on them.
        y_sbs = [None] * NQ

        def d_chains(q):
            g0 = q * QT
            msgq = msgqs[q]
            y_sb = ypool.tile([128, MD, QT], BF, tag="y_sb")
            y_sbs[q] = y_sb
            for dp in range(MD // 2):
                yps = ps_y.tile([128, 2, QT], FP32, tag="y")
                for j in range(2):
                    dt = 2 * dp + j
                    for k in range(KD):
                        nc.tensor.matmul(
                            yps[:, j, :],
                            m1_sb[:, k, dt * 128:(dt + 1) * 128],
                            xT_sb[:, k, HALO + g0:HALO + g0 + QT],
                            start=(k == 0),
                            stop=False,
                        )
                    nc.tensor.matmul(
                        yps[:, j, :],
                        ident_sb,
                        msgq[:, dt, :],
                        start=False,
                        stop=True,
                    )
                if use_merge_b:
                    for j in range(2):
                        nc.vector.tensor_scalar_add(
                            yps[:, j, :], yps[:, j, :],
                            mb_sb[:, 2 * dp + j:2 * dp + j + 1],
                        )
                nc.scalar.activation(
                    out=y_sb[:, 2 * dp:2 * dp + 2, :], in_=yps, func=AF.Copy
                )

        def d_finish(q):
            g0 = q * QT
            y_sb = y_sbs[q]
            y2_sb = ypool.tile([128, MD, QT], BF, tag="y2_sb", bufs=2)
            nc.vector.tensor_mul(y2_sb, y_sb, y_sb)
            # Sum over all 1024 features: ones-matmul reduces partitions,
            # chaining over the 8 d-tiles accumulates the rest.  Results
            # land broadcast across partitions: [:, 0:256]=sum, [256:]=sumsq.
            st = ps_log.tile([128, 512], FP32, tag="logit")
            for dt in range(MD):
                nc.tensor.matmul(
                    st[:, 0:QT], ones_sb, y_sb[:, dt, :],
                    start=(dt == 0), stop=(dt == MD - 1),
                )
            for dt in range(MD):
                nc.tensor.matmul(
                    st[:, QT:2 * QT], ones_sb, y2_sb[:, dt, :],
                    start=(dt == 0), stop=(dt == MD - 1),
                )
            mean = lnpool.tile([128, QT], FP32, tag="mean")
            nc.vector.tensor_scalar_mul(mean, st[:, 0:QT], 1.0 / D)
            veps = lnpool.tile([128, QT], FP32, tag="veps")
            nc.vector.tensor_scalar_mul(veps, st[:, QT:2 * QT], 1.0 / D)
            m2e = lnpool.tile([128, QT], FP32, tag="m2e")
            nc.vector.scalar_tensor_tensor(   # mean^2 - eps
                out=m2e, in0=mean, scalar=1.0, in1=mean,
                op0=ALU.mult, op1=ALU.mult,
            )
            nc.vector.tensor_scalar_add(m2e, m2e, -EPS)
            nc.vector.tensor_tensor(veps, veps, m2e, op=ALU.subtract)
            # rstd = rsqrt(veps) via bit-trick seed + 2 Newton steps.
            rbits = lnpool.tile([128, QT], I32, tag="rbits")
            nc.vector.tensor_scalar(
                out=rbits, in0=veps.bitcast(I32), scalar1=one_i[:, 0:1],
                scalar2=None, op0=ALU.arith_shift_right,
            )
            nc.vector.tensor_tensor(
                out=rbits, in0=magic_sb.to_broadcast([128, QT]), in1=rbits,
                op=ALU.subtract,
            )
            rstd = rbits.bitcast(FP32)
            for _ in range(2):
                nt1 = lnpool.tile([128, QT], FP32, tag="nt1")
                nc.vector.tensor_mul(nt1, rstd, rstd)
                nc.vector.tensor_mul(nt1, nt1, veps)
                nc.vector.tensor_scalar(
                    out=nt1, in0=nt1, scalar1=-0.5, scalar2=1.5,
                    op0=ALU.mult, op1=ALU.add,
                )
                nc.vector.tensor_mul(rstd, rstd, nt1)
            mean_bf = lnpool.tile([128, QT], BF, tag="mean_bf")
            nc.vector.tensor_copy(mean_bf, mean)
            rstd_bf = lnpool.tile([128, QT], BF, tag="rstd_bf")
            nc.vector.tensor_copy(rstd_bf, rstd)
            yout = ypool.tile([128, MD, QT], BF, tag="yout", bufs=2)
            nc.vector.tensor_tensor(
                yout, y_sb, bcast(mean_bf[:, :], MD), op=ALU.subtract
            )
            nc.vector.tensor_mul(yout, yout, bcast(rstd_bf[:, :], MD))
            if use_gamma_beta:
                for dt in range(MD):
                    nc.vector.tensor_scalar(
                        out=yout[:, dt, :], in0=yout[:, dt, :],
                        scalar1=gam_sb[:, dt:dt + 1],
                        scalar2=bet_sb[:, dt:dt + 1],
                        op0=ALU.mult, op1=ALU.add,
                    )
            # SWDGE store casts bf16 -> fp32 on the way to DRAM.
            nc.gpsimd.dma_start(out=y_r[:, :, g0:g0 + QT], in_=yout)

        d_chains(0)
        for q in range(1, NQ):
            d_chains(q)
            d_finish(q - 1)
        d_finish(NQ - 1)
    nc.compile()
    return nc


_CACHE: dict = {}


def _get_nc(flags):
    if flags not in _CACHE:
        _CACHE[flags] = build_nc(flags)
    return _CACHE[flags]


def kernel(x, w1, b1, w2, b2, wv_w, wv_b, merge_w, merge_b, gamma, beta):
    x = np.asarray(x, dtype=np.float32)
    w1 = np.asarray(w1, dtype=np.float32)
    b1 = np.asarray(b1, dtype=np.float32)
    w2 = np.asarray(w2, dtype=np.float32)
    b2 = np.asarray(b2, dtype=np.float32)
    wv_w = np.asarray(wv_w, dtype=np.float32)
    wv_b = np.asarray(wv_b, dtype=np.float32)
    merge_w = np.asarray(merge_w, dtype=np.float32)
    merge_b = np.asarray(merge_b, dtype=np.float32)
    gamma = np.asarray(gamma, dtype=np.float32)
    beta = np.asarray(beta, dtype=np.float32)

    m2h = 0.5 * merge_w[D:]
    wvm = wv_w @ m2h
    k2 = wv_b @ m2h
    use_gamma_beta = not (np.all(gamma == 1.0) and np.all(beta == 0.0))
    use_merge_b = bool(np.any(merge_b != 0.0))
    use_b1 = bool(np.any(b1 != 0.0))
    use_k2 = bool(np.any(k2 != 0.0))
    b2_half = 0.5 * float(b2[0])
    flags = (use_gamma_beta, use_merge_b, use_b1, use_k2, b2_half)
    nc = _get_nc(flags)

    x2 = x.reshape(B * T, D)
    shared = {
        "w1a": w1[:D].astype(BF16),
        "w1c": w1[D:].astype(BF16),
        "wvm": wvm.astype(BF16),
        "m1": merge_w[:D].astype(BF16),
        "w2rep": np.ascontiguousarray(
            np.broadcast_to(w2.reshape(H, 1), (H, 128))
        ).astype(BF16),
        "ident": np.eye(128, dtype=np.float32).astype(BF16),
    }
    if use_b1:
        shared["b1r"] = np.ascontiguousarray(b1.reshape(MH, 128).T)
    if use_k2:
        shared["k2r"] = np.ascontiguousarray(k2.reshape(MD, 128).T)
    if use_gamma_beta:
        shared["gamr"] = np.ascontiguousarray(gamma.reshape(MD, 128).T)
        shared["betr"] = np.ascontiguousarray(beta.reshape(MD, 128).T)
    if use_merge_b:
        shared["mbr"] = np.ascontiguousarray(merge_b.reshape(MD, 128).T)

    in_maps = []
    for c in range(NCORES):
        t0 = c * NTOK
        xs = np.zeros((GRID, D), np.float32)
        xs[HALO:] = x2[t0:t0 + NTOK]
        if t0 % T != 0:  # halo stays inside the same batch element
            xs[:HALO] = x2[t0 - HALO:t0]
        m = dict(shared)
        m["xT"] = np.ascontiguousarray(xs.T).astype(BF16)
        in_maps.append(m)

    res = run_bass_kernel_spmd(nc, in_maps, core_ids=list(range(NCORES)))
    out = np.concatenate([r["y"].T for r in res.results], axis=0)
    return out.reshape(B, T, D).astype(np.float32)
